# revision 32
# baseline (speedup 1.0000x reference)
"""Multi-head attention kernel for 8 Trainium2 NeuronCores.

Problem: B=2, S=2048, D=1024, H=16 heads, head_dim=64 (torch-Linear style
projections: x @ W.T + b).

Sharding: 8 cores = batch (2) x head-groups (4 heads each, 4 groups).
Each core computes, for its batch b and head slice hs..hs+256:
  QT = (w_q_slice/8) @ x_q.T + b_q_slice/8      -> [256, 2048]  (head-dim major)
  KT = w_k_slice @ x_k.T + b_k_slice            -> [256, 2048]
  V  = x_v @ w_v_slice.T + b_v_slice            -> [2048, 4*(64+1)] (ones col)
  per head h (64 rows of QT/KT, 65 cols of V):
    S.T chunk = KT_h_chunk.T @ QT_h              (scores transposed, [k,q])
    E = exp(S.T)                                 (no max subtraction)
    ctxT[0:64] += V_h65.T @ E ; ctxT[64] = rowsum(E)   (ones-column trick)
    ctxT[0:64] *= partition_broadcast(1/ctxT[64])      (gpsimd broadcast)
  out_partial = ctx @ w_o_slice.T               -> [2048, 1024] bf16
Host sums the 4 partials per batch (fp32) and adds b_o.

All matmul operands are bf16 (same 1 cycle/row PE rate as f32r, half
the DMA).  Schedule highlights:
  - DMA order consts|wqkv|xv0|xk0|xq0|xv1|xk1|xq1 (halves of each [D,S]
    input), small DMAs merged above the ~625ns per-DMA descriptor cost;
    first exp fires ~29us in.
  - One software-pipelined stream over all (q-half, head, k-chunk)
    stages: scores+exp for stage n+1 are emitted before the ctx matmuls
    of stage n so exp never waits behind ctx on the in-order PE queue.
  - Second-half KT/QT/V projections and the first q-half's output
    projection are folded into the attention stream at the (head,
    k-chunk) just before each tile's first consumer.
  - Activation runs exp only (one table load); bias adds on DVE via
    per-partition tensor_scalar; softmax normalize = DVE reciprocal +
    gpsimd partition_broadcast + DVE multiply, with the ctx PSUM rows
    spilled to SBUF so the PSUM bank frees before the next head.
  - Output tiles stream out per 128-row block (merged [128,1024] bf16
    DMAs in the tail).
(fp8e4m3 DoubleRow for the ctx matmul was tried: 2x PE but ~2.5% rms
error from E-quantization fails the 2e-2 gate.  exp needs bias=-2 to
avoid fp8 overflow if retried.)
"""

import numpy as np
import ml_dtypes

B, S, D, H, HD = 2, 2048, 1024, 16, 64
N_CORES = 8
GROUPS = 4            # head groups (cores per batch)
HPC = 4               # heads per core
DS = HPC * HD         # 256, d_model slice per core
QB = 512              # q block (matmul moving dim)
KC = S // 128         # 16 k chunks in attention
DK = D // 128         # 8 contraction chunks in projections
SH = S // 2           # 1024, sequence half

BF16 = ml_dtypes.bfloat16
_built = None


def _build(reps=1):
    import concourse.bacc as bacc
    import concourse.tile as tile
    from concourse import mybir

    F32 = mybir.dt.float32
    BF = mybir.dt.bfloat16
    Exp = mybir.ActivationFunctionType.Exp

    nc = bacc.Bacc("TRN2", target_bir_lowering=False, debug=False,
                   num_devices=N_CORES)

    xq = nc.dram_tensor("xq", [D, S], BF, kind="ExternalInput").ap()
    xk = nc.dram_tensor("xk", [D, S], BF, kind="ExternalInput").ap()
    xv = nc.dram_tensor("xv", [D, S], BF, kind="ExternalInput").ap()
    wqkv = nc.dram_tensor("wqkv", [D, 3 * DS], BF, kind="ExternalInput").ap()
    wo = nc.dram_tensor("wo", [DS, D], BF, kind="ExternalInput").ap()
    cbm = nc.dram_tensor("cbm", [128, DS + 4], F32, kind="ExternalInput").ap()
    out = nc.dram_tensor("out", [S, D], BF, kind="ExternalOutput").ap()

    with tile.TileContext(nc) as tc, \
         nc.allow_low_precision(reason="bf16 matmul operands"):
        for rep in range(reps):
            _emit(nc, tc, tile, mybir, F32, BF, Exp,
                  xq, xk, xv, wqkv, wo, cbm, out, rep=rep)
    nc.compile()
    return nc


def _emit(nc, tc, tile, mybir, F32, BF, Exp,
          xq, xk, xv, wqkv, wo, cbm, out, rep=0):
    from contextlib import ExitStack

    Identity = mybir.ActivationFunctionType.Identity
    ctx = ExitStack()
    with ctx:
        consts = ctx.enter_context(tc.tile_pool(name=f"consts{rep}", bufs=1))
        wpool = ctx.enter_context(tc.tile_pool(name=f"wpool{rep}", bufs=1))
        persist = ctx.enter_context(tc.tile_pool(name=f"persist{rep}", bufs=1))
        xp = ctx.enter_context(tc.tile_pool(name=f"xp{rep}", bufs=32))
        ppA = ctx.enter_context(
            tc.tile_pool(name=f"ppA{rep}", bufs=2, space="PSUM"))
        pss = ctx.enter_context(
            tc.tile_pool(name=f"pss{rep}", bufs=2, space="PSUM"))
        psc = ctx.enter_context(
            tc.tile_pool(name=f"psc{rep}", bufs=2, space="PSUM"))
        ep = ctx.enter_context(tc.tile_pool(name=f"ep{rep}", bufs=6))
        bcp = ctx.enter_context(tc.tile_pool(name=f"bcp{rep}", bufs=3))
        op = ctx.enter_context(tc.tile_pool(name=f"op{rep}", bufs=10))

        # ---- consts: one merged [128, 260] DMA (bvb | bk m0,m1 | bq m0,m1)
        cb_t = consts.tile([128, DS + 4], F32, name=f"cb_{rep}",
                           tag=f"cb_{rep}")
        nc.sync.dma_start(out=cb_t, in_=cbm)
        bvb_t = cb_t[:, 0:DS]
        bk_t = [cb_t[:, DS + m:DS + m + 1] for m in range(2)]
        bq_t = [cb_t[:, DS + 2 + m:DS + 3 + m] for m in range(2)]
        r_tiles = [consts.tile([128, QB], F32, name=f"r{i}_{rep}",
                               tag=f"r{i}_{rep}") for i in range(2)]

        # ---- packed qkv weights: chunk-pair tiles (transfer > the 625ns
        # per-DMA descriptor-generation floor); DMAs emitted interleaved
        # with the first xv0 chunks below so the V projection's first
        # matmuls start as early as possible ----
        wqkv2_t = [wpool.tile([128, 2, 3 * DS], BF, name=f"wqkv{p}_{rep}",
                              tag=f"wqkv{p}_{rep}") for p in range(DK // 2)]

        def dma_wqkv(p):
            nc.sync.dma_start(
                out=wqkv2_t[p], in_=wqkv[p * 256:(p + 1) * 256, :].rearrange(
                    "(b p) c -> p b c", b=2))
        wqkv_s = [wqkv2_t[kc // 2][:, kc % 2, :] for kc in range(DK)]
        wq_s = [t[:, 0:DS] for t in wqkv_s]
        wk_s = [t[:, DS:2 * DS] for t in wqkv_s]
        wv_s = [t[:, 2 * DS:3 * DS] for t in wqkv_s]
        wo_t = [wpool.tile([128, D], BF, name=f"wo{kc}_{rep}",
                           tag=f"wo{kc}_{rep}") for kc in range(2)]

        # ---- persistent activations ----
        qt_t = [persist.tile([128, S], BF, name=f"qt{m}_{rep}",
                             tag=f"qt{m}_{rep}") for m in range(2)]
        kt_t = [persist.tile([128, S], BF, name=f"kt{m}_{rep}",
                             tag=f"kt{m}_{rep}") for m in range(2)]
        v_t = [persist.tile([128, HPC * (HD + 1)], BF, name=f"v{m}_{rep}",
                            tag=f"v{m}_{rep}") for m in range(KC)]
        ctxT_t = [persist.tile([128, S], BF, name=f"ctxT{m}_{rep}",
                               tag=f"ctxT{m}_{rep}") for m in range(2)]

        # ---- x input chunk DMA (order: xv0 xk0 xq0 xk1 xv1 xq1) ----
        xch = {}

        def dma_x(which, src, nh):
            for kc in range(DK):
                t = xp.tile([128, SH], BF, name=f"x{which}{nh}{kc}_{rep}",
                            tag=f"x_{rep}")
                nc.sync.dma_start(
                    out=t, in_=src[kc * 128:(kc + 1) * 128,
                                   nh * SH:(nh + 1) * SH])
                xch[(which, nh, kc)] = t

        # interleave: wqkv pair p arrives just before xv0 chunks 2p/2p+1
        for kc in range(DK):
            if kc % 2 == 0:
                dma_wqkv(kc // 2)
            t = xp.tile([128, SH], BF, name=f"xv0{kc}_{rep}", tag=f"x_{rep}")
            nc.sync.dma_start(out=t, in_=xv[kc * 128:(kc + 1) * 128, 0:SH])
            xch[("v", 0, kc)] = t
        dma_x("k", xk, 0)
        dma_x("q", xq, 0)

        # ---- projection emitters ----
        def proj_v(m):
            nh, ms = m // 8, m % 8
            ps = ppA.tile([128, DS], F32, name=f"psV_{rep}",
                          tag=f"ppA_{rep}", padded_shape=[128, QB])
            for kc in range(DK):
                nc.tensor.matmul(
                    ps[:, :],
                    xch[("v", nh, kc)][:, ms * 128:(ms + 1) * 128],
                    wv_s[kc],
                    start=(kc == 0), stop=(kc == DK - 1))
            vm = v_t[m].rearrange("p (h c) -> p h c", h=HPC)
            nc.vector.tensor_add(
                vm[:, :, 0:HD],
                ps.rearrange("p (h c) -> p h c", h=HPC),
                bvb_t.rearrange("p (h c) -> p h c", h=HPC))
            nc.vector.memset(vm[:, :, HD:HD + 1], 1.0)

        def proj_qk(which, nh, m, n2):
            w_sb = wq_s if which == "q" else wk_s
            b_sb = bq_t if which == "q" else bk_t
            dst = qt_t if which == "q" else kt_t
            ps = ppA.tile([128, QB], F32, name=f"psP_{rep}", tag=f"ppA_{rep}")
            for kc in range(DK):
                nc.tensor.matmul(
                    ps[:, :],
                    w_sb[kc][:, m * 128:(m + 1) * 128],
                    xch[(which, nh, kc)][:, n2 * QB:(n2 + 1) * QB],
                    start=(kc == 0), stop=(kc == DK - 1))
            col = (nh * 2 + n2) * QB
            nc.vector.tensor_scalar_add(dst[m][:, col:col + QB], ps[:, :],
                                        b_sb[m])

        # ---- prologue projections ----
        for m in range(8):
            proj_v(m)
        proj_qk("k", 0, 0, 0)
        proj_qk("k", 0, 0, 1)
        proj_qk("k", 0, 1, 0)
        proj_qk("q", 0, 0, 0)
        proj_qk("q", 0, 0, 1)

        # ---- second-half x inputs: emitted after the prologue
        # projections so their semaphores sit after the prologue's
        # dependency targets (DMA device order is unchanged) ----
        dma_x("v", xv, 1)
        dma_x("k", xk, 1)
        dma_x("q", xq, 1)
        for kc in range(2):
            nc.sync.dma_start(out=wo_t[kc],
                              in_=wo[kc * 128:(kc + 1) * 128, :])

        def out_half(m, n2):
            ps = ppA.tile([128, QB], F32, name=f"psO_{rep}", tag=f"ppA_{rep}")
            for kcc in range(2):
                nc.tensor.matmul(
                    ps[:, :],
                    ctxT_t[kcc][:, m * 128:(m + 1) * 128],
                    wo_t[kcc][:, n2 * QB:(n2 + 1) * QB],
                    start=(kcc == 0), stop=(kcc == 1))
            ot = op.tile([128, QB], BF, name=f"oth_{rep}", tag=f"oth_{rep}")
            nc.vector.tensor_copy(ot, ps[:, :])
            nc.sync.dma_start(
                out=out[m * 128:(m + 1) * 128, n2 * QB:(n2 + 1) * QB],
                in_=ot)

        # ---- phase C emitter: one m-block = 2 psum tiles (alternating
        # pools), 2 copies into one [128, D] tile, 1 merged DMA ----
        def out_block(m, use_act=False):
            ot = op.tile([128, D], BF, name=f"ot_{rep}", tag=f"ot_{rep}")
            for n2 in range(2):
                pool = ppA if n2 == 0 else pss
                ps = pool.tile(
                    [128, QB], F32, name=f"psO_{rep}",
                    tag=(f"ppA_{rep}" if pool is ppA else f"pss_{rep}"),
                    padded_shape=[128, 2 * QB] if pool is pss else None)
                for kcc in range(2):
                    nc.tensor.matmul(
                        ps[:, :],
                        ctxT_t[kcc][:, m * 128:(m + 1) * 128],
                        wo_t[kcc][:, n2 * QB:(n2 + 1) * QB],
                        start=(kcc == 0), stop=(kcc == 1))
                if use_act and n2 == 1:
                    nc.scalar.activation(ot[:, n2 * QB:(n2 + 1) * QB],
                                         ps[:, :], Identity)
                else:
                    nc.vector.tensor_copy(ot[:, n2 * QB:(n2 + 1) * QB],
                                          ps[:, :])
            nc.sync.dma_start(out=out[m * 128:(m + 1) * 128, :], in_=ot)

        # ---- attention ----
        # One software-pipelined stream over all (qp, h, kc) stages: the
        # scores matmuls + exp for stage n+1 are emitted BEFORE the ctx
        # matmuls of stage n, so the next head's exp never waits behind the
        # previous head's ctx accumulation on the in-order PE queue.  The
        # normalize spills ctx PSUM rows 0:65 to SBUF first (single cheap
        # reader) so the PSUM slot frees before the next head needs it.
        spool = ctx.enter_context(tc.tile_pool(name=f"sp{rep}", bufs=3))
        it = 0
        ctx_ps = {}

        def emit_ctx(h, kc, e_sb):
            for j in range(2):
                nc.tensor.matmul(
                    ctx_ps[h][j][0:HD + 1, :],
                    v_t[kc][:, h * (HD + 1):(h + 1) * (HD + 1)],
                    e_sb[:, j, :],
                    start=(kc == 0), stop=(kc == KC - 1))

        def emit_norm_j(qp, h, j, spill=True, fine=False):
            nonlocal it
            ti, ro = h // 2, (h % 2) * 64
            qb = qp * 2 + j
            if spill:
                src = spool.tile([HD + 1, QB], F32, name=f"cs_{rep}",
                                 tag=f"cs_{rep}")
                nc.vector.tensor_copy(src, ctx_ps[h][j][0:HD + 1, :])
            else:
                src = ctx_ps[h][j]
            rt = r_tiles[it % 2]
            it += 1
            nc.vector.reciprocal(rt[0:1, :], src[HD:HD + 1, :])
            bsb = bcp.tile([64, QB], F32, name=f"bsb_{rep}",
                           tag=f"bsb_{rep}")
            nc.gpsimd.partition_broadcast(bsb, rt[0:1, :], channels=64)
            if fine:
                # 128-col pieces so each tail out-block unblocks asap
                for c in range(0, QB, 128):
                    nc.vector.tensor_mul(
                        ctxT_t[ti][ro:ro + 64,
                                   qb * QB + c:qb * QB + c + 128],
                        src[0:HD, c:c + 128], bsb[:, c:c + 128])
            else:
                nc.vector.tensor_mul(
                    ctxT_t[ti][ro:ro + 64, qb * QB:(qb + 1) * QB],
                    src[0:HD, :], bsb)

        def emit_norm(qp, h):
            for j in range(2):
                emit_norm_j(qp, h, j)

        P = proj_qk
        hooks = {
            # qp0: fold second-half projections in, deadline-ordered.
            (0, 0, 3): [lambda: proj_v(8)],
            (0, 0, 4): [lambda: proj_v(9)],
            (0, 0, 5): [lambda: proj_v(10)],
            (0, 0, 6): [lambda: proj_v(11)],
            (0, 0, 7): [lambda: P("k", 1, 0, 0)],
            (0, 0, 9): [lambda: proj_v(12)],
            (0, 0, 10): [lambda: proj_v(13)],
            (0, 0, 11): [lambda: P("k", 1, 0, 1)],
            (0, 0, 12): [lambda: proj_v(14)],
            (0, 0, 13): [lambda: proj_v(15)],
            (0, 1, 2): [lambda: P("k", 0, 1, 1)],
            (0, 1, 6): [lambda: P("q", 0, 1, 0)],
            (0, 1, 10): [lambda: P("q", 0, 1, 1)],
            (0, 2, 2): [lambda: P("k", 1, 1, 0)],
            (0, 2, 6): [lambda: P("k", 1, 1, 1)],
            (0, 3, 2): [lambda: P("q", 1, 0, 0)],
            (0, 3, 6): [lambda: P("q", 1, 0, 1)],
            # qp1: QT second half for heads 2-3.
            (1, 1, 1): [lambda: P("q", 1, 1, 0)],
            (1, 1, 9): [lambda: P("q", 1, 1, 1)],
        }
        # qp1: fold qp0's output projection in (DVE copies only -- Act must
        # keep its Exp table).
        slots = [(0, 3), (0, 7), (0, 11), (0, 14), (1, 3), (1, 6), (1, 12),
                 (1, 14), (2, 2), (2, 5), (2, 8), (2, 11), (2, 14), (3, 2),
                 (3, 5), (3, 8)]
        for i, (h, kcs) in enumerate(slots):
            m, n2 = i // 2, i % 2
            hooks.setdefault((1, h, kcs), []).append(
                (lambda mm, nn: (lambda: out_half(mm, nn)))(m, n2))

        seq = [(qp, h, kc)
               for qp in range(2) for h in range(HPC) for kc in range(KC)]
        pend = None  # previous stage whose ctx matmuls are not yet emitted
        for qp, h, kc in seq:
            for fn in hooks.get((qp, h, kc), ()):
                fn()
            if kc == 0:
                ctx_ps[h] = [psc.tile([128, QB], F32, name=f"ctxps_{rep}",
                                      tag=f"psc_{rep}") for _ in range(2)]
            ti, ro = h // 2, (h % 2) * 64
            qh = qt_t[ti][ro:ro + 64, :]
            kh = kt_t[ti][ro:ro + 64, :]
            sps = pss.tile([128, 2, QB], F32, name=f"sps_{rep}",
                           tag=f"pss_{rep}")
            for j in range(2):
                qb = qp * 2 + j
                nc.tensor.matmul(
                    sps[:, j, :],
                    kh[:, kc * 128:(kc + 1) * 128],
                    qh[:, qb * QB:(qb + 1) * QB],
                    start=True, stop=True)
            e_sb = ep.tile([128, 2, QB], BF, name=f"e_{rep}", tag=f"e_{rep}")
            nc.scalar.activation(e_sb, sps[:, :, :], Exp)
            if pend is not None:
                pqp, ph, pkc, pe = pend
                emit_ctx(ph, pkc, pe)
                if pkc == KC - 1:
                    emit_norm(pqp, ph)
                    del ctx_ps[ph]
            pend = (qp, h, kc, e_sb)
        pqp, ph, pkc, pe = pend
        emit_ctx(ph, pkc, pe)
        # Last head: normalize straight from PSUM (no next-head WAR to
        # protect), and emit the qb2 output units between the two
        # normalizes so they start as soon as their columns are final.
        emit_norm_j(pqp, ph, 0, spill=False, fine=True)
        for m in range(8, 12):
            out_block(m, use_act=True)
        emit_norm_j(pqp, ph, 1, spill=False, fine=True)
        del ctx_ps[ph]
        for m in range(12, KC):
            out_block(m, use_act=True)


def _in_maps(q, k, v, w_q, b_q, w_k, b_k, w_v, b_v, w_o):
    scale = 1.0 / np.sqrt(HD)
    wqT = np.ascontiguousarray(w_q.T * scale)      # [D, D]
    wkT = np.ascontiguousarray(w_k.T)
    wvT = np.ascontiguousarray(w_v.T)
    maps = []
    for c in range(N_CORES):
        b, g = c // GROUPS, c % GROUPS
        hs = g * DS
        wqkv = np.concatenate(
            [wqT[:, hs:hs + DS], wkT[:, hs:hs + DS], wvT[:, hs:hs + DS]],
            axis=1)
        cbm = np.zeros((128, DS + 4), np.float32)
        cbm[:, 0:DS] = np.broadcast_to(b_v[hs:hs + DS], (128, DS))
        cbm[:, DS + 0] = b_k[hs:hs + 128]
        cbm[:, DS + 1] = b_k[hs + 128:hs + DS]
        cbm[:, DS + 2] = b_q[hs:hs + 128] * scale
        cbm[:, DS + 3] = b_q[hs + 128:hs + DS] * scale
        maps.append({
            "xq": np.ascontiguousarray(q[b].T).astype(BF16),
            "xk": np.ascontiguousarray(k[b].T).astype(BF16),
            "xv": np.ascontiguousarray(v[b].T).astype(BF16),
            "wqkv": np.ascontiguousarray(wqkv).astype(BF16),
            "wo": np.ascontiguousarray(w_o[:, hs:hs + DS].T).astype(BF16),
            "cbm": cbm,
        })
    return maps


def kernel(q, k, v, w_q, b_q, w_k, b_k, w_v, b_v, w_o, b_o):
    global _built
    arrs = [np.asarray(a, dtype=np.float32)
            for a in (q, k, v, w_q, b_q, w_k, b_k, w_v, b_v, w_o)]
    q, k, v, w_q, b_q, w_k, b_k, w_v, b_v, w_o = arrs
    b_o = np.asarray(b_o, dtype=np.float32)
    if _built is None:
        _built = _build()
    from concourse import bass2jax
    results = bass2jax.run_bass_via_pjrt(
        _built, _in_maps(q, k, v, w_q, b_q, w_k, b_k, w_v, b_v, w_o),
        n_cores=N_CORES)
    o = np.zeros((B, S, D), np.float32)
    for c in range(N_CORES):
        o[c // GROUPS] += np.asarray(results[c]["out"], dtype=np.float32)
    o += b_o
    return o


# revision 34
# speedup vs baseline: 2.3669x; 2.3669x over previous
"""Multi-head attention kernel for 8 Trainium2 NeuronCores.

Problem: B=2, S=2048, D=1024, H=16 heads, head_dim=64 (torch-Linear style
projections: x @ W.T + b).

Sharding: 8 cores = batch (2) x head-groups (4 heads each, 4 groups).
Each core computes, for its batch b and head slice hs..hs+256:
  QT = (w_q_slice/8) @ x_q.T + b_q_slice/8      -> [256, 2048]  (head-dim major)
  KT = w_k_slice @ x_k.T + b_k_slice            -> [256, 2048]
  V  = x_v @ w_v_slice.T + b_v_slice            -> [2048, 4*(64+1)] (ones col)
  per head h (64 rows of QT/KT, 65 cols of V):
    S.T chunk = KT_h_chunk.T @ QT_h              (scores transposed, [k,q])
    E = exp(S.T)                                 (no max subtraction)
    ctxT[0:64] += V_h65.T @ E ; ctxT[64] = rowsum(E)   (ones-column trick)
    ctxT[0:64] *= partition_broadcast(1/ctxT[64])      (gpsimd broadcast)
  out_partial = ctx @ w_o_slice.T               -> [2048, 1024] bf16
Host sums the 4 partials per batch (fp32) and adds b_o.

All matmul operands are bf16 (same 1 cycle/row PE rate as f32r, half
the DMA).  Schedule highlights:
  - DMA order consts|wqkv|xv0|xk0|xq0|xv1|xk1|xq1 (halves of each [D,S]
    input), small DMAs merged above the ~625ns per-DMA descriptor cost;
    first exp fires ~29us in.
  - One software-pipelined stream over all (q-half, head, k-chunk)
    stages: scores+exp for stage n+1 are emitted before the ctx matmuls
    of stage n so exp never waits behind ctx on the in-order PE queue.
  - Second-half KT/QT/V projections and the first q-half's output
    projection are folded into the attention stream at the (head,
    k-chunk) just before each tile's first consumer.
  - Activation runs exp only (one table load); bias adds on DVE via
    per-partition tensor_scalar; softmax normalize = DVE reciprocal +
    gpsimd partition_broadcast + DVE multiply, with the ctx PSUM rows
    spilled to SBUF so the PSUM bank frees before the next head.
  - Output tiles stream out per 128-row block (merged [128,1024] bf16
    DMAs in the tail).
(fp8e4m3 DoubleRow for the ctx matmul was tried: 2x PE but ~2.5% rms
error from E-quantization fails the 2e-2 gate.  exp needs bias=-2 to
avoid fp8 overflow if retried.)
"""

import numpy as np
import ml_dtypes

B, S, D, H, HD = 2, 2048, 1024, 16, 64
N_CORES = 8
GROUPS = 4            # head groups (cores per batch)
HPC = 4               # heads per core
DS = HPC * HD         # 256, d_model slice per core
QB = 512              # q block (matmul moving dim)
KC = S // 128         # 16 k chunks in attention
DK = D // 128         # 8 contraction chunks in projections
SH = S // 2           # 1024, sequence half

BF16 = ml_dtypes.bfloat16
_built = None


def _build(reps=1):
    import concourse.bacc as bacc
    import concourse.tile as tile
    from concourse import mybir

    F32 = mybir.dt.float32
    BF = mybir.dt.bfloat16
    Exp = mybir.ActivationFunctionType.Exp

    nc = bacc.Bacc("TRN2", target_bir_lowering=False, debug=False,
                   num_devices=N_CORES)

    xq = nc.dram_tensor("xq", [D, S], BF, kind="ExternalInput").ap()
    xk = nc.dram_tensor("xk", [D, S], BF, kind="ExternalInput").ap()
    xv = nc.dram_tensor("xv", [D, S], BF, kind="ExternalInput").ap()
    wqkv = nc.dram_tensor("wqkv", [D, 3 * DS], BF, kind="ExternalInput").ap()
    wo = nc.dram_tensor("wo", [DS, D], BF, kind="ExternalInput").ap()
    cbm = nc.dram_tensor("cbm", [128, DS + 4], F32, kind="ExternalInput").ap()
    out = nc.dram_tensor("out", [S, D], BF, kind="ExternalOutput").ap()

    with tile.TileContext(nc) as tc, \
         nc.allow_low_precision(reason="bf16 matmul operands"):
        for rep in range(reps):
            _emit(nc, tc, tile, mybir, F32, BF, Exp,
                  xq, xk, xv, wqkv, wo, cbm, out, rep=rep)
    nc.compile()
    return nc


def _emit(nc, tc, tile, mybir, F32, BF, Exp,
          xq, xk, xv, wqkv, wo, cbm, out, rep=0):
    from contextlib import ExitStack

    Identity = mybir.ActivationFunctionType.Identity
    ctx = ExitStack()
    with ctx:
        consts = ctx.enter_context(tc.tile_pool(name=f"consts{rep}", bufs=1))
        wpool = ctx.enter_context(tc.tile_pool(name=f"wpool{rep}", bufs=1))
        persist = ctx.enter_context(tc.tile_pool(name=f"persist{rep}", bufs=1))
        xp = ctx.enter_context(tc.tile_pool(name=f"xp{rep}", bufs=32))
        ppA = ctx.enter_context(
            tc.tile_pool(name=f"ppA{rep}", bufs=2, space="PSUM"))
        pss = ctx.enter_context(
            tc.tile_pool(name=f"pss{rep}", bufs=2, space="PSUM"))
        psc = ctx.enter_context(
            tc.tile_pool(name=f"psc{rep}", bufs=2, space="PSUM"))
        ep = ctx.enter_context(tc.tile_pool(name=f"ep{rep}", bufs=6))
        bcp = ctx.enter_context(tc.tile_pool(name=f"bcp{rep}", bufs=3))
        op = ctx.enter_context(tc.tile_pool(name=f"op{rep}", bufs=10))

        # ---- consts: one merged [128, 260] DMA (bvb | bk m0,m1 | bq m0,m1)
        cb_t = consts.tile([128, DS + 4], F32, name=f"cb_{rep}",
                           tag=f"cb_{rep}")
        nc.sync.dma_start(out=cb_t, in_=cbm)
        bvb_t = cb_t[:, 0:DS]
        bk_t = [cb_t[:, DS + m:DS + m + 1] for m in range(2)]
        bq_t = [cb_t[:, DS + 2 + m:DS + 3 + m] for m in range(2)]
        r_tiles = [consts.tile([128, QB], F32, name=f"r{i}_{rep}",
                               tag=f"r{i}_{rep}") for i in range(2)]

        # ---- packed qkv weights: chunk-pair tiles (transfer > the 625ns
        # per-DMA descriptor-generation floor); DMAs emitted interleaved
        # with the first xv0 chunks below so the V projection's first
        # matmuls start as early as possible ----
        wqkv2_t = [wpool.tile([128, 2, 3 * DS], BF, name=f"wqkv{p}_{rep}",
                              tag=f"wqkv{p}_{rep}") for p in range(DK // 2)]

        def dma_wqkv(p):
            nc.sync.dma_start(
                out=wqkv2_t[p], in_=wqkv[p * 256:(p + 1) * 256, :].rearrange(
                    "(b p) c -> p b c", b=2))
        wqkv_s = [wqkv2_t[kc // 2][:, kc % 2, :] for kc in range(DK)]
        wq_s = [t[:, 0:DS] for t in wqkv_s]
        wk_s = [t[:, DS:2 * DS] for t in wqkv_s]
        wv_s = [t[:, 2 * DS:3 * DS] for t in wqkv_s]
        wo_t = [wpool.tile([128, D], BF, name=f"wo{kc}_{rep}",
                           tag=f"wo{kc}_{rep}") for kc in range(2)]

        # ---- persistent activations ----
        qt_t = [persist.tile([128, S], BF, name=f"qt{m}_{rep}",
                             tag=f"qt{m}_{rep}") for m in range(2)]
        kt_t = [persist.tile([128, S], BF, name=f"kt{m}_{rep}",
                             tag=f"kt{m}_{rep}") for m in range(2)]
        v_t = [persist.tile([128, HPC * (HD + 1)], BF, name=f"v{m}_{rep}",
                            tag=f"v{m}_{rep}") for m in range(KC)]
        ctxT_t = [persist.tile([128, S], BF, name=f"ctxT{m}_{rep}",
                               tag=f"ctxT{m}_{rep}") for m in range(2)]

        # ---- x input chunk DMA (order: xv0 xk0 xq0 xk1 xv1 xq1) ----
        xch = {}

        def dma_x(which, src, nh):
            for kc in range(DK):
                t = xp.tile([128, SH], BF, name=f"x{which}{nh}{kc}_{rep}",
                            tag=f"x_{rep}")
                nc.sync.dma_start(
                    out=t, in_=src[kc * 128:(kc + 1) * 128,
                                   nh * SH:(nh + 1) * SH])
                xch[(which, nh, kc)] = t

        # interleave: wqkv pair p arrives just before xv0 chunks 2p/2p+1
        for kc in range(DK):
            if kc % 2 == 0:
                dma_wqkv(kc // 2)
            t = xp.tile([128, SH], BF, name=f"xv0{kc}_{rep}", tag=f"x_{rep}")
            nc.sync.dma_start(out=t, in_=xv[kc * 128:(kc + 1) * 128, 0:SH])
            xch[("v", 0, kc)] = t
        dma_x("k", xk, 0)
        dma_x("q", xq, 0)

        # ---- projection emitters ----
        def proj_v(m):
            nh, ms = m // 8, m % 8
            ps = ppA.tile([128, DS], F32, name=f"psV_{rep}",
                          tag=f"ppA_{rep}", padded_shape=[128, QB])
            for kc in range(DK):
                nc.tensor.matmul(
                    ps[:, :],
                    xch[("v", nh, kc)][:, ms * 128:(ms + 1) * 128],
                    wv_s[kc],
                    start=(kc == 0), stop=(kc == DK - 1))
            vm = v_t[m].rearrange("p (h c) -> p h c", h=HPC)
            nc.vector.tensor_add(
                vm[:, :, 0:HD],
                ps.rearrange("p (h c) -> p h c", h=HPC),
                bvb_t.rearrange("p (h c) -> p h c", h=HPC))
            nc.vector.memset(vm[:, :, HD:HD + 1], 1.0)

        def proj_qk(which, nh, m, n2):
            w_sb = wq_s if which == "q" else wk_s
            b_sb = bq_t if which == "q" else bk_t
            dst = qt_t if which == "q" else kt_t
            ps = ppA.tile([128, QB], F32, name=f"psP_{rep}", tag=f"ppA_{rep}")
            for kc in range(DK):
                nc.tensor.matmul(
                    ps[:, :],
                    w_sb[kc][:, m * 128:(m + 1) * 128],
                    xch[(which, nh, kc)][:, n2 * QB:(n2 + 1) * QB],
                    start=(kc == 0), stop=(kc == DK - 1))
            col = (nh * 2 + n2) * QB
            nc.vector.tensor_scalar_add(dst[m][:, col:col + QB], ps[:, :],
                                        b_sb[m])

        # ---- prologue projections ----
        for m in range(8):
            proj_v(m)
        proj_qk("k", 0, 0, 0)
        proj_qk("k", 0, 0, 1)
        proj_qk("k", 0, 1, 0)
        proj_qk("q", 0, 0, 0)
        proj_qk("q", 0, 0, 1)

        # ---- second-half x inputs: emitted after the prologue
        # projections so their semaphores sit after the prologue's
        # dependency targets (DMA device order is unchanged) ----
        dma_x("v", xv, 1)
        dma_x("k", xk, 1)
        dma_x("q", xq, 1)
        for kc in range(2):
            nc.sync.dma_start(out=wo_t[kc],
                              in_=wo[kc * 128:(kc + 1) * 128, :])

        def out_half(m, n2):
            ps = ppA.tile([128, QB], F32, name=f"psO_{rep}", tag=f"ppA_{rep}")
            for kcc in range(2):
                nc.tensor.matmul(
                    ps[:, :],
                    ctxT_t[kcc][:, m * 128:(m + 1) * 128],
                    wo_t[kcc][:, n2 * QB:(n2 + 1) * QB],
                    start=(kcc == 0), stop=(kcc == 1))
            ot = op.tile([128, QB], BF, name=f"oth_{rep}", tag=f"oth_{rep}")
            nc.vector.tensor_copy(ot, ps[:, :])
            nc.sync.dma_start(
                out=out[m * 128:(m + 1) * 128, n2 * QB:(n2 + 1) * QB],
                in_=ot)

        # ---- phase C emitter: one m-block = 2 psum tiles (alternating
        # pools), 2 copies into one [128, D] tile, 1 merged DMA ----
        def out_block(m, use_act=False):
            ot = op.tile([128, D], BF, name=f"ot_{rep}", tag=f"ot_{rep}")
            for n2 in range(2):
                pool = ppA if n2 == 0 else pss
                ps = pool.tile(
                    [128, QB], F32, name=f"psO_{rep}",
                    tag=(f"ppA_{rep}" if pool is ppA else f"pss_{rep}"),
                    padded_shape=[128, 2 * QB] if pool is pss else None)
                for kcc in range(2):
                    nc.tensor.matmul(
                        ps[:, :],
                        ctxT_t[kcc][:, m * 128:(m + 1) * 128],
                        wo_t[kcc][:, n2 * QB:(n2 + 1) * QB],
                        start=(kcc == 0), stop=(kcc == 1))
                if use_act and n2 == 1:
                    nc.scalar.activation(ot[:, n2 * QB:(n2 + 1) * QB],
                                         ps[:, :], Identity)
                else:
                    nc.vector.tensor_copy(ot[:, n2 * QB:(n2 + 1) * QB],
                                          ps[:, :])
            nc.sync.dma_start(out=out[m * 128:(m + 1) * 128, :], in_=ot)

        # ---- attention ----
        # One software-pipelined stream over all (qp, h, kc) stages: the
        # scores matmuls + exp for stage n+1 are emitted BEFORE the ctx
        # matmuls of stage n, so the next head's exp never waits behind the
        # previous head's ctx accumulation on the in-order PE queue.  The
        # normalize spills ctx PSUM rows 0:65 to SBUF first (single cheap
        # reader) so the PSUM slot frees before the next head needs it.
        spool = ctx.enter_context(tc.tile_pool(name=f"sp{rep}", bufs=3))
        it = 0
        ctx_ps = {}

        def emit_ctx(h, kc, e_sb):
            for j in range(2):
                nc.tensor.matmul(
                    ctx_ps[h][j][0:HD + 1, :],
                    v_t[kc][:, h * (HD + 1):(h + 1) * (HD + 1)],
                    e_sb[:, j, :],
                    start=(kc == 0), stop=(kc == KC - 1))

        def emit_norm_j(qp, h, j, spill=True, fine=False):
            nonlocal it
            ti, ro = h // 2, (h % 2) * 64
            qb = qp * 2 + j
            if spill:
                src = spool.tile([HD + 1, QB], F32, name=f"cs_{rep}",
                                 tag=f"cs_{rep}")
                nc.vector.tensor_copy(src, ctx_ps[h][j][0:HD + 1, :])
            else:
                src = ctx_ps[h][j]
            rt = r_tiles[it % 2]
            it += 1
            nc.vector.reciprocal(rt[0:1, :], src[HD:HD + 1, :])
            bsb = bcp.tile([64, QB], F32, name=f"bsb_{rep}",
                           tag=f"bsb_{rep}")
            nc.gpsimd.partition_broadcast(bsb, rt[0:1, :], channels=64)
            if fine:
                # 128-col pieces so each tail out-block unblocks asap
                for c in range(0, QB, 128):
                    nc.vector.tensor_mul(
                        ctxT_t[ti][ro:ro + 64,
                                   qb * QB + c:qb * QB + c + 128],
                        src[0:HD, c:c + 128], bsb[:, c:c + 128])
            else:
                nc.vector.tensor_mul(
                    ctxT_t[ti][ro:ro + 64, qb * QB:(qb + 1) * QB],
                    src[0:HD, :], bsb)

        def emit_norm(qp, h):
            for j in range(2):
                emit_norm_j(qp, h, j)

        P = proj_qk
        hooks = {
            # qp0: fold second-half projections in, deadline-ordered.
            (0, 0, 3): [lambda: proj_v(8)],
            (0, 0, 4): [lambda: proj_v(9)],
            (0, 0, 5): [lambda: proj_v(10)],
            (0, 0, 6): [lambda: proj_v(11)],
            (0, 0, 7): [lambda: P("k", 1, 0, 0)],
            (0, 0, 9): [lambda: proj_v(12)],
            (0, 0, 10): [lambda: proj_v(13)],
            (0, 0, 11): [lambda: P("k", 1, 0, 1)],
            (0, 0, 12): [lambda: proj_v(14)],
            (0, 0, 13): [lambda: proj_v(15)],
            (0, 1, 2): [lambda: P("k", 0, 1, 1)],
            (0, 1, 6): [lambda: P("q", 0, 1, 0)],
            (0, 1, 10): [lambda: P("q", 0, 1, 1)],
            (0, 2, 2): [lambda: P("k", 1, 1, 0)],
            (0, 2, 6): [lambda: P("k", 1, 1, 1)],
            (0, 3, 2): [lambda: P("q", 1, 0, 0)],
            (0, 3, 6): [lambda: P("q", 1, 0, 1)],
            # qp1: QT second half for heads 2-3.
            (1, 1, 1): [lambda: P("q", 1, 1, 0)],
            (1, 1, 9): [lambda: P("q", 1, 1, 1)],
        }
        # qp1: fold qp0's output projection in (DVE copies only -- Act must
        # keep its Exp table).
        slots = [(0, 3), (0, 7), (0, 11), (0, 14), (1, 3), (1, 6), (1, 12),
                 (1, 14), (2, 2), (2, 5), (2, 8), (2, 11), (2, 14), (3, 2),
                 (3, 5), (3, 8)]
        for i, (h, kcs) in enumerate(slots):
            m, n2 = i // 2, i % 2
            hooks.setdefault((1, h, kcs), []).append(
                (lambda mm, nn: (lambda: out_half(mm, nn)))(m, n2))

        seq = [(qp, h, kc)
               for qp in range(2) for h in range(HPC) for kc in range(KC)]
        pend = None  # previous stage whose ctx matmuls are not yet emitted
        for qp, h, kc in seq:
            for fn in hooks.get((qp, h, kc), ()):
                fn()
            if kc == 0:
                ctx_ps[h] = [psc.tile([128, QB], F32, name=f"ctxps_{rep}",
                                      tag=f"psc_{rep}") for _ in range(2)]
            ti, ro = h // 2, (h % 2) * 64
            qh = qt_t[ti][ro:ro + 64, :]
            kh = kt_t[ti][ro:ro + 64, :]
            sps = pss.tile([128, 2, QB], F32, name=f"sps_{rep}",
                           tag=f"pss_{rep}")
            for j in range(2):
                qb = qp * 2 + j
                nc.tensor.matmul(
                    sps[:, j, :],
                    kh[:, kc * 128:(kc + 1) * 128],
                    qh[:, qb * QB:(qb + 1) * QB],
                    start=True, stop=True)
            e_sb = ep.tile([128, 2, QB], BF, name=f"e_{rep}", tag=f"e_{rep}")
            nc.scalar.activation(e_sb, sps[:, :, :], Exp)
            if pend is not None:
                pqp, ph, pkc, pe = pend
                emit_ctx(ph, pkc, pe)
                if pkc == KC - 1:
                    emit_norm(pqp, ph)
                    del ctx_ps[ph]
            pend = (qp, h, kc, e_sb)
        pqp, ph, pkc, pe = pend
        emit_ctx(ph, pkc, pe)
        # Last head: normalize straight from PSUM (no next-head WAR to
        # protect), and emit the qb2 output units between the two
        # normalizes so they start as soon as their columns are final.
        emit_norm_j(pqp, ph, 0, spill=False, fine=True)
        for m in range(8, 12):
            out_block(m, use_act=True)
        emit_norm_j(pqp, ph, 1, spill=False, fine=True)
        del ctx_ps[ph]
        for m in range(12, KC):
            out_block(m, use_act=True)


def _in_maps(q, k, v, w_q, b_q, w_k, b_k, w_v, b_v, w_o):
    scale = 1.0 / np.sqrt(HD)
    wqT = np.ascontiguousarray(w_q.T * scale)      # [D, D]
    wkT = np.ascontiguousarray(w_k.T)
    wvT = np.ascontiguousarray(w_v.T)
    maps = []
    for c in range(N_CORES):
        b, g = c // GROUPS, c % GROUPS
        hs = g * DS
        wqkv = np.concatenate(
            [wqT[:, hs:hs + DS], wkT[:, hs:hs + DS], wvT[:, hs:hs + DS]],
            axis=1)
        cbm = np.zeros((128, DS + 4), np.float32)
        cbm[:, 0:DS] = np.broadcast_to(b_v[hs:hs + DS], (128, DS))
        cbm[:, DS + 0] = b_k[hs:hs + 128]
        cbm[:, DS + 1] = b_k[hs + 128:hs + DS]
        cbm[:, DS + 2] = b_q[hs:hs + 128] * scale
        cbm[:, DS + 3] = b_q[hs + 128:hs + DS] * scale
        maps.append({
            "xq": np.ascontiguousarray(q[b].T).astype(BF16),
            "xk": np.ascontiguousarray(k[b].T).astype(BF16),
            "xv": np.ascontiguousarray(v[b].T).astype(BF16),
            "wqkv": np.ascontiguousarray(wqkv).astype(BF16),
            "wo": np.ascontiguousarray(w_o[:, hs:hs + DS].T).astype(BF16),
            "cbm": cbm,
        })
    return maps


def kernel(q, k, v, w_q, b_q, w_k, b_k, w_v, b_v, w_o, b_o):
    global _built
    arrs = [np.asarray(a, dtype=np.float32)
            for a in (q, k, v, w_q, b_q, w_k, b_k, w_v, b_v, w_o)]
    q, k, v, w_q, b_q, w_k, b_k, w_v, b_v, w_o = arrs
    b_o = np.asarray(b_o, dtype=np.float32)
    if _built is None:
        _built = _build()
    from concourse import bass2jax
    results = bass2jax.run_bass_via_pjrt(
        _built, _in_maps(q, k, v, w_q, b_q, w_k, b_k, w_v, b_v, w_o),
        n_cores=N_CORES)
    o = np.zeros((B, S, D), np.float32)
    for c in range(N_CORES):
        o[c // GROUPS] += np.asarray(results[c]["out"], dtype=np.float32)
    o += b_o
    return o
